# revision 25
# baseline (speedup 1.0000x reference)
"""Trainium2 Bass kernel for nn_CustomAttention — fp8 DoubleRow Gram restructure.

Math: scores_h = Wq_h (X X^T) Wk_h^T + rank-1 bias terms + RoPE(s<32) delta.
The Gram X X^T is shared across all 16 heads AND q/k.

v2 over the fp16 baseline: the two heavy streams (Gram 54.5us, V-proj 54.5us
of PE) move to fp8e4 DoubleRow matmuls (0.5 cycles/row, 2 k-tiles per step =
4x fp16 row throughput), with a two-word operand split to keep precision:
  h16 = e4m3(16*x), l16 = e4m3(16*(x - h16/16))     (~7.2 significand bits)
All products are computed at scale 256 = 16*16 in one psum (hh + h*l + l*h for
the Gram; Wh*h + Wl*h + Wh*l for V), so the split costs no extra evictions:
the 1/256 folds into the existing Gram eviction scale, and for V into the
host-side bias (bv*256) and the 1/rowsum factor (rp/256). Dropped 4th-order
terms (l*l) are ~1e-3 of values; measured end-to-end rel err ~1.3e-2 vs the
2e-2 budget (emulated in exp_num2.py; fp16 baseline was 8.4e-3).

The Gram block is also cut by symmetry: G[c',c]=G[c,c'] in LOCAL tile indices
on both cores (the odd-core column swap is perm-consistent), so only the 26
lower-triangle [128,128] tiles are computed (vs 32); the 6 upper tiles are PE
transposes of evicted lower tiles. PE: Gram 33.3+0.3us, V 41us (12 DoubleRow
steps/unit vs 8 fp16 steps), B/S/O unchanged fp16 (13.6/1.7/6.8us) ~= 98us.

Sharding: 8 cores = 4 batches x 2 column-halves; partial scores summed via
pairwise ReduceScatter hidden behind V. Host precomputes the rank-1 bias
terms + RoPE delta ('fix', a pure input function, 0.3% of FLOPs) and the
dtype splits/layouts (elementwise reformatting only).

Schedule: xs pair-stream (h+l) on alternating SP/ACT queues through Gram; wk
woven into late Gram, wq/wv/xt through B/S; V chunks straddle the collective;
out chunks interleave into the V tail (same proven order as the baseline).
"""
import numpy as np
import ml_dtypes

import concourse.bacc as bacc
import concourse.mybir as mybir
import concourse.tile as tile
from concourse.bass_utils import run_bass_kernel_spmd

F32 = mybir.dt.float32
F16 = mybir.dt.float16
E4 = mybir.dt.float8e4
E4NP = ml_dtypes.float8_e4m3
DR = mybir.MatmulPerfMode.DoubleRow

B, S, D = 4, 4096, 1024
H, DH = 16, 64
HPC = 8             # heads per core (post-RS)
ROT = 32
THETA = 10000.0
P = 128
NSP = 16            # s-tile PAIRS for the Gram stream (32 s-tiles)
SC = 512            # s-chunk for v / out
NSC = S // SC       # 8
CT = D // P         # 8 c-tiles
NPAIR = HPC // 2    # 4 local head pairs
SCALE = DH ** -0.5
CBLK = D // 2       # 512-column Gram block per core
PPRE = 7            # pair prefetch depth
QS = 16.0           # operand quant scale; psum scale = QS*QS = 256


def build_nc():
    nc = bacc.Bacc(trn_type="TRN2", num_devices=8)

    # packed h/l streams: one DMA per fetch keeps queue-issue count low
    xg = nc.dram_tensor("xg", [S, 2, D], E4, kind="ExternalInput")      # natural
    xtg = nc.dram_tensor("xtg", [D, NSC, 2, SC], E4, kind="ExternalInput")  # transposed, h/l per chunk
    wkT = nc.dram_tensor("wkT", [D, H * DH], F16, kind="ExternalInput")
    wqT = nc.dram_tensor("wqT", [CBLK, H * DH], F16, kind="ExternalInput")
    wvg = nc.dram_tensor("wvg", [D, 2, HPC * DH], E4, kind="ExternalInput")
    bv = nc.dram_tensor("bv", [P, NPAIR], F32, kind="ExternalInput")    # 256*bv
    fix = nc.dram_tensor("fix", [DH, HPC * DH], F32, kind="ExternalInput")
    ident = nc.dram_tensor("ident", [DH, DH], F16, kind="ExternalInput")
    ident128 = nc.dram_tensor("ident128", [P, P], F16, kind="ExternalInput")
    ident32 = nc.dram_tensor("ident32", [DH, DH], F32, kind="ExternalInput")
    out = nc.dram_tensor("out", [NPAIR, P, S], F16, kind="ExternalOutput")

    outr = out.rearrange("j p s -> p j s")
    xgr = xg.rearrange("(sp two p) hl c -> p sp two hl c", p=P, two=2)
    xtgr = xtg.rearrange("(ct p) sc hl s -> p ct sc hl s", p=P)
    wkTr = wkT.rearrange("(kt p) e -> p kt e", p=P)
    wqTr = wqT.rearrange("(kt p) d -> p kt d", p=P)
    wvgr = wvg.rearrange("(ct p) hl f -> p ct hl f", p=P)

    with tile.TileContext(nc) as tc:
        with (
            tc.tile_pool(name="singles", bufs=1) as singles,
            tc.tile_pool(name="xgp", bufs=PPRE + 1) as xgp,
            tc.tile_pool(name="xtp", bufs=NSC) as xtp,
            tc.tile_pool(name="otp", bufs=4) as otp,
            tc.tile_pool(name="ps", bufs=8, space="PSUM") as ps,
            tc.tile_pool(name="dram", bufs=1, space="DRAM") as dram,
        ):
            # ---- xs pair stream: tile [P, lane(2), hl(2), D] ----
            xg_tiles = []

            def pair_fetch(sp, split=False):
                a = xgp.tile([P, 2, 2, D], E4, tag="xg", name=f"xg{sp}")
                q = nc.sync if sp % 2 == 0 else nc.scalar
                if split and sp == 0:
                    # fine pieces split across BOTH queues, landing in PE
                    # consumption order (mt0 needs just cols 0:128 of h+l)
                    nc.sync.dma_start(a[:, :, 0, 0:P], xgr[:, sp, :, 0, 0:P])
                    nc.scalar.dma_start(a[:, :, 1, 0:P], xgr[:, sp, :, 1, 0:P])
                    nc.sync.dma_start(a[:, :, 0, P:CBLK], xgr[:, sp, :, 0, P:CBLK])
                    nc.scalar.dma_start(a[:, :, 1, P:CBLK], xgr[:, sp, :, 1, P:CBLK])
                    nc.sync.dma_start(a[:, :, 0, CBLK:D], xgr[:, sp, :, 0, CBLK:D])
                    nc.scalar.dma_start(a[:, :, 1, CBLK:D], xgr[:, sp, :, 1, CBLK:D])
                elif split:
                    nc.sync.dma_start(a[:, :, 0, :], xgr[:, sp, :, 0, :])
                    nc.scalar.dma_start(a[:, :, 1, :], xgr[:, sp, :, 1, :])
                else:
                    q.dma_start(a, xgr[:, sp, :, :, :])
                xg_tiles.append(a)

            pair_fetch(0, split=True)
            pair_fetch(1, split=True)
            for sp in range(2, PPRE):
                pair_fetch(sp)

            # small tables (fetched post-Gram via the gated block)
            bv_sb = singles.tile([P, NPAIR], F32)
            fix_sb = singles.tile([DH, HPC * DH], F32)
            id_sb = singles.tile([DH, DH], F16)
            id128_sb = singles.tile([P, P], F16)
            id32_sb = singles.tile([DH, DH], F32)
            bd_tiles = [singles.tile([P, P], F16, name=f"bd{jj}")
                        for jj in range(NPAIR)]
            for bd in bd_tiles:
                nc.vector.memset(bd, 0.0)

            wk_sb = singles.tile([P, CT, H * DH], F16)
            wq_sb = singles.tile([P, CBLK // P, H * DH], F16)
            wv_sb = singles.tile([P, CT, 2, HPC * DH], E4)

            xt_tiles = []

            def xt_fetch(sc, q=None, gate_src=None):
                # a 2-elem DVE copy from late-arriving data gates the
                # transfer (WAW on the dest): admission control vs the
                # global DMA FIFO, which otherwise runs the transfer as soon
                # as its queue issues it and starves the Gram xs stream
                a = xtp.tile([P, CT, 2, SC], E4, tag="xt", name=f"xt{sc}")
                if gate_src is not None:
                    nc.vector.tensor_copy(a[0:1, 0, 0, 0:2], gate_src)
                (q or nc.sync).dma_start(a, xtgr[:, :, sc, :, :])
                xt_tiles.append(a)

            cc_in = dram.tile([2, DH, HPC * DH], F32)
            cc_out = dram.tile([DH, HPC * DH], F32)

            vt = singles.tile([P, NPAIR, S], F16)

            # eviction engine alternation
            evict_i = [0]

            def evict_copy(dst, src, scale=None):
                e = "dve" if evict_i[0] % 2 == 0 else "act"
                if e == "dve":
                    if scale is None:
                        nc.vector.tensor_copy(dst, src)
                    else:
                        nc.vector.tensor_scalar_mul(dst, src, scale)
                else:
                    if scale is None:
                        nc.scalar.activation(
                            dst, src, mybir.ActivationFunctionType.Copy)
                    else:
                        nc.scalar.activation(
                            dst, src, mybir.ActivationFunctionType.Copy,
                            scale=scale)
                evict_i[0] += 1

            # ==== phase G: lower-triangle Gram at psum scale 256 ====
            # local tile symmetry (mt,cc)<->(cc,mt) holds on both cores: the
            # odd-core column swap maps local cols 0..3 to global tiles 4..7
            # and local rows likewise (perm-consistent), so one SPMD program.
            gps = [ps.tile([P, CBLK], F32, tag="ps", name=f"gps{mt}")
                   for mt in range(CT)]

            def gate(region, gate_src, q, dst, srcap):
                # 2-elem DVE copy whose RAW dep on gate_src holds the DMA
                # back (WAW) until that data exists; engine wait-queues let a
                # plain memset fire early, a data-dep copy cannot
                nc.vector.tensor_copy(region, gate_src)
                q.dma_start(dst, srcap)

            def side_dma(sp):
                # all weight/table transfers gated on the LAST pair's arrival:
                # they fill the DMA tail slack behind the stream, ahead of B/S
                if sp != 10:
                    return
                g15 = xg_tiles[15][0:1, 0, 0, 0:2]
                gate(wk_sb[0:1, 2:3, 0:2], g15, nc.scalar,
                     wk_sb[:, 2:4, :], wkTr[:, 2:4, :])
                gate(wk_sb[0:1, 4:5, 0:2], g15, nc.sync,
                     wk_sb[:, 4:6, :], wkTr[:, 4:6, :])
                gate(wk_sb[0:1, 6:7, 0:2], g15, nc.scalar,
                     wk_sb[:, 6:8, :], wkTr[:, 6:8, :])
                gate(wk_sb[0:1, 0:1, 0:2], g15, nc.sync,
                     wk_sb[:, 0:2, :], wkTr[:, 0:2, :])
                gate(wq_sb[0:1, 0:1, 0:2], g15, nc.sync,
                     wq_sb[:, 0:2, :], wqTr[:, 0:2, :])
                gate(wq_sb[0:1, 2:3, 0:2], g15, nc.sync,
                     wq_sb[:, 2:4, :], wqTr[:, 2:4, :])
                gate(wv_sb[0:1, 0, 0, 0:2], g15, nc.scalar,
                     wv_sb, wvgr[:, :, :, :])
                gate(id128_sb[0:1, 0:2], g15, nc.scalar,
                     id128_sb, ident128[:, :])
                gate(id_sb[0:1, 0:2], g15, nc.scalar, id_sb, ident[:, :])
                gate(id32_sb[0:1, 0:2], g15, nc.scalar,
                     id32_sb, ident32[:, :])
                gate(bv_sb[0:1, 0:2], g15, nc.scalar, bv_sb, bv[:, :])
                gate(fix_sb[0:1, 0:2], g15, nc.scalar, fix_sb, fix[:, :])

            for sp in range(NSP):
                if sp + PPRE < NSP:
                    pair_fetch(sp + PPRE)
                side_dma(sp)
                xgt = xg_tiles[sp]
                first, last = sp == 0, sp == NSP - 1

                def g_step(mt, hl_w, hl_i, st, sp_):
                    n = min((mt + 1) * P, CBLK)
                    ms = slice(mt * P, (mt + 1) * P)
                    nc.tensor.matmul(
                        gps[mt][:, 0:n], xgt[:, :, hl_w, ms],
                        xgt[:, :, hl_i, 0:n],
                        start=(sp_ == 0 and st == 0),
                        stop=(sp_ == NSP - 1 and st == 2),
                        perf_mode=DR, skip_group_check=True)

                if sp < 2:
                    # hh pass first: consumes only the h pieces, which land
                    # ahead of the l pieces during warmup
                    for mt in range(CT):
                        g_step(mt, 0, 0, 0, sp)
                    for mt in range(CT):
                        g_step(mt, 0, 1, 1, sp)
                    for mt in range(CT):
                        g_step(mt, 1, 0, 2, sp)
                elif sp == NSP - 1:
                    # last pair: finish mt rows in eviction order so the
                    # first evictions (and B) start before the row-0 tail
                    for mt in (3, 4, 5, 6, 7, 1, 2, 0):
                        g_step(mt, 0, 0, 0, sp)
                        g_step(mt, 0, 1, 1, sp)
                        g_step(mt, 1, 0, 2, sp)
                else:
                    for mt in range(CT):
                        g_step(mt, 0, 0, 0, sp)
                        g_step(mt, 0, 1, 1, sp)
                        g_step(mt, 1, 0, 2, sp)

            # evictions (x 1/256) in B's consumption order, DVE-only, with
            # the deferred weight/xt DMAs gated between them (DVE memset =
            # WAW gate): their transfers otherwise jump ahead of the xs
            # stream in the global DMA FIFO and starve the Gram phase.
            gsb = singles.tile([P, CT, CBLK], F16)

            def g_evict(mt):
                n = min((mt + 1) * P, CBLK)
                nc.vector.tensor_scalar_mul(
                    gsb[:, mt, 0:n], gps[mt][:, 0:n], 1.0 / (QS * QS))

            gps3 = gps[3][0:1, 0:2]
            xt_fetch(0, q=nc.scalar, gate_src=gps3)
            xt_fetch(1, q=nc.sync, gate_src=gps3)
            for mt in (3, 4, 5, 6, 7, 1, 2, 0):
                g_evict(mt)

            def g_mirrors():
                # 6 upper-triangle tiles = PE transposes of evicted lower ones
                for cc in range(1, CBLK // P):
                    for mt in range(cc):
                        ptp = ps.tile([P, P], F16, tag="ps", name=f"gm{mt}_{cc}")
                        nc.tensor.transpose(
                            ptp, gsb[:, cc, mt * P:(mt + 1) * P], id128_sb)
                        nc.vector.tensor_copy(
                            gsb[:, mt, cc * P:(cc + 1) * P], ptp)

            # ---- V chunk / O chunk emitters ----
            def v_unit(sc, ft, eng=None):
                pv = ps.tile([P, SC], F32, tag="ps", name=f"pv{sc}_{ft}")
                xtc = xt_tiles[sc]
                fs = slice(ft * P, (ft + 1) * P)
                for k in range(CT // 2):
                    nc.tensor.matmul(
                        pv, wv_sb[:, 2 * k:2 * k + 2, 0, fs],
                        xtc[:, 2 * k:2 * k + 2, 0, :],
                        start=(k == 0), stop=False, perf_mode=DR)
                for k in range(CT // 2):
                    nc.tensor.matmul(
                        pv, wv_sb[:, 2 * k:2 * k + 2, 1, fs],
                        xtc[:, 2 * k:2 * k + 2, 0, :],
                        start=False, stop=False, perf_mode=DR)
                for k in range(CT // 2):
                    nc.tensor.matmul(
                        pv, wv_sb[:, 2 * k:2 * k + 2, 0, fs],
                        xtc[:, 2 * k:2 * k + 2, 1, :],
                        start=False, stop=(k == CT // 2 - 1), perf_mode=DR)
                dst = vt[:, ft, sc * SC:(sc + 1) * SC]
                use_dve = (evict_i[0] % 2 == 0) if eng is None else (eng == "dve")
                if use_dve:
                    nc.vector.tensor_scalar_add(dst, pv, bv_sb[:, ft:ft + 1])
                else:
                    nc.scalar.activation(
                        dst, pv, mybir.ActivationFunctionType.Identity,
                        bias=bv_sb[:, ft:ft + 1])
                evict_i[0] += 1

            def v_chunk(sc, eng=None):
                for ft in range(NPAIR):
                    v_unit(sc, ft, eng=eng)

            otc_tiles = {}

            def o_unit(sc, jj, eng=None):
                po = ps.tile([P, SC], F32, tag="ps", name=f"po{sc}_{jj}")
                nc.tensor.matmul(
                    po, bd_tiles[jj], vt[:, jj, sc * SC:(sc + 1) * SC],
                    start=True, stop=True)
                if sc not in otc_tiles:
                    otc_tiles[sc] = otp.tile(
                        [P, NPAIR, SC], F16, tag="ot", name=f"ot{sc}")
                dst = otc_tiles[sc][:, jj, :]
                use_dve = (evict_i[0] % 2 == 0) if eng is None else (eng == "dve")
                if use_dve:
                    nc.vector.tensor_scalar_mul(dst, po, rp[:, jj:jj + 1])
                else:
                    nc.scalar.activation(
                        dst, po, mybir.ActivationFunctionType.Copy,
                        scale=rp[:, jj:jj + 1])
                evict_i[0] += 1
                sl = slice(sc * SC, (sc + 1) * SC)
                if sc >= NSC - 2:
                    # tail chunks ship in pair-halves right after their
                    # evictions (final chunk-6 halves ship singly so the
                    # very last transfer is only 128KB)
                    if sc == NSC - 2 and jj >= 2:
                        nc.sync.dma_start(outr[:, jj:jj + 1, sl],
                                          otc_tiles[sc][:, jj:jj + 1, :])
                    elif jj in (1, NPAIR - 1):
                        nc.sync.dma_start(
                            outr[:, jj - 1:jj + 1, sl],
                            otc_tiles[sc][:, jj - 1:jj + 1, :])
                elif jj == NPAIR - 1:
                    nc.sync.dma_start(outr[:, :, sl], otc_tiles[sc])

            def o_chunk(sc):
                for jj in range(NPAIR):
                    o_unit(sc, jj)

            # ==== phase B: B[my c, e] = sum_c' G[c', c] Wk^T[c', e] ====
            # kt rows 0..2 read mirror regions of gsb; do them last
            kt_order = [3, 4, 5, 6, 7, 0, 1, 2]
            bsb = singles.tile([P, CBLK // P, H * DH], F16)
            for half in range(2):
                pbs = [ps.tile([P, CBLK], F32, tag="ps", name=f"pb{half}_{i}")
                       for i in range(4)]
                for ki, kt in enumerate(kt_order):
                    if half == 0 and ki == 5:
                        g_mirrors()  # rows 1-3 evicted by now; kts 0-2 need them
                    for mt2 in range(2):
                        lhs = gsb[:, kt, (half * 2 + mt2) * P:(half * 2 + mt2 + 1) * P]
                        for nh in range(2):
                            nc.tensor.matmul(
                                pbs[mt2 * 2 + nh], lhs,
                                wk_sb[:, kt, nh * 512:(nh + 1) * 512],
                                start=(ki == 0), stop=(ki == CT - 1),
                                skip_group_check=True,
                            )
                for mt2 in range(2):
                    for nh in range(2):
                        evict_copy(
                            bsb[:, half * 2 + mt2, nh * 512:(nh + 1) * 512],
                            pbs[mt2 * 2 + nh])
                if half == 0:
                    xt_fetch(2, q=nc.scalar, gate_src=gps3)

            # ==== phase S: partial scores, all 16 heads, pair-packed ====
            ssb = singles.tile([DH, H * DH], F32)
            for p in range(H // 2):
                sps = ps.tile([P, P], F32, tag="ps", name=f"sps{p}")
                for kt in range(CBLK // P):
                    nc.tensor.matmul(
                        sps,
                        wq_sb[:, kt, p * P:(p + 1) * P],
                        bsb[:, kt, p * P:(p + 1) * P],
                        start=(kt == 0), stop=(kt == CBLK // P - 1),
                    )
                evict_copy(ssb[:, (2 * p) * DH:(2 * p + 1) * DH],
                           sps[0:DH, 0:DH])
                evict_copy(ssb[:, (2 * p + 1) * DH:(2 * p + 2) * DH],
                           sps[DH:P, DH:P])
                if p == 3:
                    nc.sync.dma_start(cc_in[0], ssb[:, 0:HPC * DH])
                elif p == 7:
                    nc.sync.dma_start(cc_in[1], ssb[:, HPC * DH:])

            # ---- pairwise ReduceScatter: even core keeps heads 0-7 ----
            nc.gpsimd.collective_compute(
                "ReduceScatter",
                mybir.AluOpType.add,
                replica_groups=[[0, 1], [2, 3], [4, 5], [6, 7]],
                ins=[cc_in.opt()],
                outs=[cc_out.opt()],
            )
            srs = singles.tile([DH, HPC * DH], F32)
            nc.sync.dma_start(srs, cc_out[:])

            # ==== V straddles the collective ====
            v_chunk(0)
            v_chunk(1)
            # xt3..7 transfers run during the collective: gated on the last
            # scores eviction so they enqueue just AFTER the cc planes
            gsrs = ssb[0:1, H * DH - 2:H * DH]
            for sc_ in range(3, NSC):
                xt_fetch(sc_, q=nc.scalar, gate_src=gsrs)
            v_chunk(2)
            v_chunk(3)
            v_chunk(4)
            v_chunk(5)

            # softmax (placed after V5 in program order so the V evictions
            # precede exp on ACT and psum banks recycle without waiting srs)
            sfx = singles.tile([DH, HPC * DH], F32)
            nc.vector.tensor_add(sfx, srs, fix_sb)
            mx = singles.tile([DH, HPC], F32)
            nc.vector.reduce_max(
                mx, sfx.rearrange("p (h e) -> p h e", e=DH),
                axis=mybir.AxisListType.X, negate=True)
            sfm = singles.tile([DH, HPC * DH], F32)
            nc.vector.tensor_tensor(
                sfm.rearrange("p (h e) -> p h e", e=DH),
                sfx.rearrange("p (h e) -> p h e", e=DH),
                mx[:, :, None].to_broadcast((DH, HPC, DH)),
                mybir.AluOpType.add)
            pn = singles.tile([DH, HPC * DH], F16)
            nc.scalar.activation(
                pn, sfm, mybir.ActivationFunctionType.Exp, scale=SCALE)
            sums = singles.tile([DH, HPC], F32)
            nc.vector.reduce_sum(
                sums, pn.rearrange("p (h e) -> p h e", e=DH),
                axis=mybir.AxisListType.X)
            rec = singles.tile([DH, HPC], F32)
            nc.vector.reciprocal(rec, sums)

            # probsT tiles
            for jj in range(NPAIR):
                pt_ps = ps.tile([P, DH], F16, tag="ps", name=f"pt{jj}")
                nc.tensor.transpose(pt_ps, pn[:, jj * P:(jj + 1) * P], id_sb)
                bd = bd_tiles[jj]
                nc.vector.tensor_copy(bd[0:DH, 0:DH], pt_ps[0:DH, :])
                nc.vector.tensor_copy(bd[DH:P, DH:P], pt_ps[DH:P, :])

            # rp[p, jj] = (1/256)/rowsum for out partition p
            rec3 = rec.rearrange("p (j two) -> p j two", two=2)
            rp = singles.tile([P, NPAIR], F32)
            nc.vector.tensor_scalar_mul(rp[0:DH, :], rec3[:, :, 0], 1.0 / (QS * QS))
            rp_ps = ps.tile([P, SC], F32, tag="ps", name="rp_ps")
            nc.tensor.matmul(
                rp_ps[DH:P, 0:NPAIR], id32_sb, rec3[:, :, 1],
                start=True, stop=True, tile_position=(0, 64),
            )
            nc.vector.tensor_scalar_mul(rp[DH:P, :], rp_ps[DH:P, 0:NPAIR],
                                        1.0 / (QS * QS))

            # ==== remaining V + O interleave; fine-grained tail ====
            o_chunk(0)
            v_chunk(6)
            o_chunk(1)
            o_chunk(2)
            v_unit(7, 0)
            o_unit(3, 0)
            o_unit(3, 1)
            v_unit(7, 1)
            o_unit(7, 0)
            o_unit(3, 2)
            o_unit(3, 3)
            v_unit(7, 2)
            o_unit(7, 1)
            o_unit(4, 0)
            o_unit(4, 1)
            o_unit(4, 2)
            o_unit(4, 3)
            o_unit(5, 0)
            o_unit(5, 1)
            o_unit(5, 2)
            o_unit(5, 3)
            v_unit(7, 3)
            o_unit(7, 2, eng="act")
            o_unit(6, 0, eng="dve")
            o_unit(6, 1, eng="act")
            o_unit(7, 3, eng="dve")
            o_unit(6, 2, eng="act")
            o_unit(6, 3, eng="dve")

    nc.finalize()
    return nc


def _host_fix(x, Wq, Wk, bq, bk):
    """Rank-1 bias terms + RoPE(s<32) delta, per batch, fp64 on host.
    Returns (B, H, 64, 64) fp32."""
    inv_freq = 1.0 / (THETA ** (np.arange(0, ROT, 2, dtype=np.float64) / ROT))
    d_idx = np.arange(DH, dtype=np.float64)
    fr = d_idx[:, None] * inv_freq[np.repeat(np.arange(ROT // 2), 2)][None, :]
    cos_t, sin_t = np.cos(fr), np.sin(fr)

    def rope32(t):
        rh = np.empty_like(t)
        rh[..., 0::2] = -t[..., 1::2]
        rh[..., 1::2] = t[..., 0::2]
        return t * cos_t[None] + rh * sin_t[None]

    Wq64, Wk64 = Wq.astype(np.float64), Wk.astype(np.float64)
    bq64, bk64 = bq.astype(np.float64), bk.astype(np.float64)
    fixes = np.empty((B, H, DH, DH), dtype=np.float32)
    for bb in range(B):
        Xf = x[bb].T.astype(np.float64)          # (D, S)
        gx = Xf.sum(axis=1)
        uq = np.einsum('hdc,c->hd', Wq64, gx)
        uk = np.einsum('hec,c->he', Wk64, gx)
        f = (uq[:, :, None] * bk64[:, None, :]
             + bq64[:, :, None] * uk[:, None, :]
             + S * bq64[:, :, None] * bk64[:, None, :])
        q32 = np.einsum('hdc,cs->hds', Wq64, Xf[:, :ROT]) + bq64[:, :, None]
        k32 = np.einsum('hec,cs->hes', Wk64, Xf[:, :ROT]) + bk64[:, :, None]
        f += (np.einsum('hds,hes->hde', rope32(q32), rope32(k32))
              - np.einsum('hds,hes->hde', q32, k32))
        fixes[bb] = f.astype(np.float32)
    return fixes


def _split8(a32, scale=QS):
    """a32 -> (h, l) fp8e4m3 with h = e4(scale*a), l = e4(scale*(a - h/scale))."""
    h = (a32 * scale).astype(E4NP)
    l = ((a32 - h.astype(np.float32) / scale) * scale).astype(E4NP)
    return h, l


def kernel(x, W, b):
    x = np.asarray(x, dtype=np.float32)
    W = np.asarray(W, dtype=np.float32)
    b = np.asarray(b, dtype=np.float32)

    Wr = W.reshape(H, 3, DH, D)
    br = b.reshape(H, 3, DH)
    Wq, Wk, Wv = Wr[:, 0], Wr[:, 1], Wr[:, 2]
    bq, bk = br[:, 0], br[:, 1]

    fixes = _host_fix(x, Wq, Wk, bq, bk)

    # weights, shared / per-half.  On odd cores the two 512-col blocks of
    # xs are swapped (so "my" Gram block is always cols 0:512); wkT rows
    # are swapped consistently since they pair with gsb's c' order.
    wk_flat = Wk.reshape(H * DH, D)
    wkT_c = np.ascontiguousarray(wk_flat.T).astype(np.float16)
    wkT_s = np.ascontiguousarray(
        np.concatenate([wkT_c[CBLK:], wkT_c[:CBLK]], axis=0))
    wq_flat = Wq.reshape(H * DH, D)
    wqT_full = np.ascontiguousarray(wq_flat.T).astype(np.float16)
    shard = {}
    for j in range(2):
        hs_ = slice(j * HPC, (j + 1) * HPC)
        wv8 = Wv[hs_].reshape(HPC * DH, D)
        wvT = np.ascontiguousarray(wv8.T).astype(np.float32)   # (D, 512)
        wvh8, wvl8 = _split8(wvT)
        wvg = np.ascontiguousarray(np.stack([wvh8, wvl8], axis=1))  # (D,2,512)
        bvj = (br[hs_, 2].reshape(NPAIR, 2 * DH).T * (QS * QS)).astype(np.float32)
        wqT_j = np.ascontiguousarray(wqT_full[j * CBLK:(j + 1) * CBLK])
        shard[j] = (wvg, bvj, wqT_j)

    xg8, xg8s, xtg8 = [], [], []
    for bb in range(B):
        a = np.ascontiguousarray(x[bb])
        h8, l8 = _split8(a)
        xg8.append(np.ascontiguousarray(np.stack([h8, l8], axis=1)))  # (S,2,D)
        xg8s.append(np.ascontiguousarray(np.stack(
            [np.concatenate([h8[:, CBLK:], h8[:, :CBLK]], axis=1),
             np.concatenate([l8[:, CBLK:], l8[:, :CBLK]], axis=1)], axis=1)))
        xtg8.append(np.ascontiguousarray(np.stack(
            [h8.T.reshape(D, NSC, SC), l8.T.reshape(D, NSC, SC)],
            axis=2)))                                                 # (D,NSC,2,SC)
    ident = np.eye(DH, dtype=np.float16)
    ident128 = np.eye(P, dtype=np.float16)
    ident32 = np.eye(DH, dtype=np.float32)

    nc = build_nc()
    in_maps = []
    for core in range(8):
        bb, j = core // 2, core % 2
        wvg, bvj, wqT_j = shard[j]
        fx = fixes[bb, j * HPC:(j + 1) * HPC]
        fx = np.ascontiguousarray(fx.transpose(1, 0, 2).reshape(DH, HPC * DH))
        in_maps.append({
            "xg": xg8[bb] if j == 0 else xg8s[bb],
            "xtg": xtg8[bb],
            "wkT": wkT_c if j == 0 else wkT_s,
            "wqT": wqT_j,
            "wvg": wvg, "bv": bvj, "fix": fx,
            "ident": ident, "ident128": ident128, "ident32": ident32,
        })

    res = run_bass_kernel_spmd(nc, in_maps, core_ids=list(range(8)))

    # reference's transpose+reshape = C-order reinterpret of (H, dh, B, S)
    big = np.empty((H, DH, B, S), dtype=np.float32)
    for core in range(8):
        bb, j = core // 2, core % 2
        oc = res.results[core]["out"].reshape(NPAIR, 2, DH, S)
        for jj in range(NPAIR):
            for half in range(2):
                big[j * HPC + 2 * jj + half, :, bb, :] = oc[jj, half]
    return big.reshape(B, S, D)
